# revision 29
# baseline (speedup 1.0000x reference)
"""Trainium2 Bass kernel for causal single-head attention with QKV projections.

Problem shape: B=4, S=4096, E=512, H=64 (fp32 inputs, causal mask).

Strategy (8 NeuronCores, data-parallel):
  - core j handles batch j%4; half j//4 of that batch's query rows.
    Half 0 = 512-row blocks {0,1,6,7}, half 1 = {2,3,4,5}: both halves do
    exactly 72 causal key-chunks of score work and 40 softmax rounds, and
    half 1 only ever reads K/V columns [0, 3072).
  - Host pre-transposes Q/K/V slabs to [E, S] layout and casts to bf16 so all
    device matmuls have the contraction dim on partitions.
  - On device: project Qt=[H,Sq], Kt=[H,S], vTp=[H,S] (fp32 biases added
    exactly via per-partition bias) with 1024-wide moving operands, then
    flash-style causal attention with scores kept transposed [k-part, q-free]:
       St = Kt_chunk^T @ Qt  -> exp fused into the PSUM->SBUF evacuation
       O^T (+denominator row) = [v | 1]^T @ P accumulated in PSUM
    No max-subtraction (scores are bounded, |s|<1 after 1/sqrt(E) scaling, so
    exp is safe; softmax is shift-invariant so the result matches reference).
  - The head dim (H=64) is duplicated into partitions 64..127 via
    host-duplicated projection weights so paired streams run their score
    matmuls concurrently in disjoint PE row groups (tile_position row tiling),
    each contracting K=64 exactly, and the PE clock stays un-throttled.
  - All transposes run on the DMA xbar (dma transpose), not the PE:
    v_sb [k,h] chunks come from vTp via sync-engine xbar transposes, and the
    per-512-block epilogue transposes O^T via a bf16 xbar round trip before
    the denominator divide.  PSUM holds exactly 8 banks: proj psum (2) +
    2x double-buffered score super-tile (4) + the pair's two O^T
    accumulators (2).
  - Streams are paired by similar causal length ((0,1) then (2,3)); score
    matmuls run one 1024-col K/V group ahead of the PV matmuls with st/pv
    emission interleaved so the PE never waits on the exp evacuations.
  - Input HBM traffic is issued as 1 MiB chunks, q/k first, inside each
    variant branch; the causal-mask diag multiplies and v_sb memsets run on
    the otherwise-idle GPSIMD engine.
"""

import sys

sys.path.insert(0, "/opt/trn_rl_repo")

import math

import numpy as np
import ml_dtypes

B, S, E, H = 4, 4096, 512, 64
N_CORES = 8
SQ = S // 2  # 2048 query rows per core
JBLK = 512  # query block size
NJ = SQ // JBLK  # 4 query blocks per core
KCH = 128  # key chunk size
JGLOBALS = [[0, 1, 6, 7], [2, 3, 4, 5]]  # global 512-row q-block ids per half
KVEXT = [8, 6]  # 512-col K/V blocks each variant actually reads
SCALE = 1.0 / math.sqrt(float(E))

BF16 = ml_dtypes.bfloat16

_CACHE = {}


def _build():
    import concourse.mybir as mybir
    from concourse import bacc, tile

    f32 = mybir.dt.float32
    bf16 = mybir.dt.bfloat16

    nc = bacc.Bacc(
        "TRN2", target_bir_lowering=False, debug=False, num_devices=N_CORES
    )

    f8 = mybir.dt.float8e4
    i16 = mybir.dt.int16

    # Inputs are host-packed to [128 partitions, chunk(1024 cols), e-chunk,
    # col] order so every input DMA is 128 contiguous runs (cheap issue).
    qT = nc.dram_tensor("qT", [128, 4 * SQ], f8, kind="ExternalInput")
    kT = nc.dram_tensor("kT", [128, 4 * S], f8, kind="ExternalInput")
    vT = nc.dram_tensor("vT", [128, 4 * S], bf16, kind="ExternalInput")
    w8 = nc.dram_tensor("w8", [2, E, 2 * H], f8, kind="ExternalInput")
    wv = nc.dram_tensor("wv", [E, 2 * H], bf16, kind="ExternalInput")
    bqkv = nc.dram_tensor("bqkv", [3, 2 * H, 1], f32, kind="ExternalInput")
    out = nc.dram_tensor("out", [SQ, H], f32, kind="ExternalOutput")

    # Embedded constants: causal block mask (allowed = k <= q) and identity.
    tril_np = np.triu(np.ones((KCH, KCH), np.float32)).astype(BF16)
    trilc = nc.inline_tensor(tril_np, name="trilc")
    ident_np = np.eye(128, dtype=BF16)
    identbc = nc.inline_tensor(ident_np, name="identbc")

    with tile.TileContext(nc) as tc:
        pid = nc.partition_id()
        with (
            tc.tile_pool(name="cpool", bufs=1) as cpool,
            tc.tile_pool(name="ipool", bufs=1) as ipool,
        ):
            # ---- constants (small, issued first on the sync queue) ----
            w8_sb = cpool.tile([128, 2, 4, 2 * H], f8, name="w8_sb")
            nc.sync.dma_start(
                w8_sb[:], w8.ap().rearrange("m (c p) h -> p m c h", p=128)
            )
            wv_sb = cpool.tile([128, 4, 2 * H], bf16, name="wv_sb")
            nc.sync.dma_start(
                wv_sb[:], wv.ap().rearrange("(c p) h -> p c h", p=128)
            )
            b_sb = cpool.tile([2 * H, 3], f32, name="b_sb")
            nc.sync.dma_start(b_sb[:], bqkv.ap().rearrange("m h one -> h (m one)"))
            tril_sb = cpool.tile([KCH, KCH], bf16, name="tril_sb")
            nc.sync.dma_start(tril_sb[:], trilc.ap())
            identb_sb = cpool.tile([128, 128], bf16, name="identb_sb")
            nc.sync.dma_start(identb_sb[:], identbc.ap())
            zbias = cpool.tile([128, 1], f32, name="zbias")
            nc.vector.memset(zbias[:], 0.0)

            # ---- input tiles ([128, chunk, e-chunk, 1024] layout) ----
            qT_sb = ipool.tile([128, 2, 4, 1024], f8, name="qT_sb", tag="qT")
            kT_sb = ipool.tile([128, 4, 4, 1024], f8, name="kT_sb", tag="kT")
            vT_sb = ipool.tile([128, 4, 4, 1024], bf16, name="vT_sb", tag="vT")

            def dma_in(eng, srcd, dst, ch):
                # one 1024-col chunk = 128 contiguous 4KB-8KB runs
                eng.dma_start(
                    dst[:, ch], srcd.ap()[:, 4096 * ch : 4096 * (ch + 1)]
                )

            # q/k chunks issue on the scalar HWDGE queue (idle until the
            # first exp), v chunks on sync, so compute starts earliest;
            # the shared chunks are emitted before the variant branch
            nc.scalar.dma_start(qT_sb[:], qT.ap())
            for ch in range(3):
                dma_in(nc.scalar, kT, kT_sb, ch)
            for ch in range(3):
                dma_in(nc.sync, vT, vT_sb, ch)

            def body(jglobals, vtag):
                nblk = KVEXT[vtag]  # K/V extent in 512-col blocks
                kvhi = 512 * nblk
                if nblk > 6:
                    dma_in(nc.scalar, kT, kT_sb, 3)
                    dma_in(nc.sync, vT, vT_sb, 3)

                with (
                    tc.tile_pool(name=f"bpool{vtag}", bufs=1) as bpool,
                    tc.tile_pool(name=f"bps{vtag}", bufs=1, space="PSUM") as bps,
                ):
                    Qt = bpool.tile([2 * H, SQ], bf16, name=f"Qt{vtag}")
                    Kt = bpool.tile([2 * H, S], bf16, name=f"Kt{vtag}")
                    vTp = bpool.tile([2 * H, S], bf16, name=f"vTp{vtag}")
                    v_sb = bpool.tile(
                        [128, S // KCH, 128], bf16, name=f"v_sb{vtag}"
                    )
                    # ones column = softmax denominator row; zero padding so
                    # O^T rows H+1..79 (copied for the xbar transpose) are 0
                    nc.gpsimd.memset(v_sb[:, :, H : H + 1], 1.0)
                    nc.gpsimd.memset(v_sb[:, :, H + 1 :], 0.0)

                    def proj_block(dst, src_sb, m, blk):
                        ps = bps.tile(
                            [2 * H, 512], f32, name=f"pj{vtag}_{m}_{blk}",
                            tag="proj", bufs=2,
                        )
                        ch, off = blk // 2, 512 * (blk % 2)
                        if m < 2:
                            # fp8 DoubleRow: each matmul contracts a pair of
                            # 128-row E-chunks (2 fp8 weights per PE cell)
                            for g in range(2):
                                nc.tensor.matmul(
                                    ps[:],
                                    w8_sb[:, m, 2 * g : 2 * g + 2, :],
                                    src_sb[:, ch, 2 * g : 2 * g + 2, off : off + 512],
                                    start=(g == 0),
                                    stop=(g == 1),
                                    perf_mode=mybir.MatmulPerfMode.DoubleRow,
                                )
                        else:
                            for c in range(4):
                                nc.tensor.matmul(
                                    ps[:],
                                    wv_sb[:, c, :],
                                    src_sb[:, ch, c, off : off + 512],
                                    start=(c == 0),
                                    stop=(c == 3),
                                )
                        nc.vector.tensor_scalar_add(
                            dst[:, 512 * blk : 512 * blk + 512], ps[:],
                            b_sb[:, m : m + 1],
                        )

                    def proj_group(dst, src_sb, m, lo, hi):
                        for blk in range(lo // 512, hi // 512):
                            proj_block(dst, src_sb, m, blk)

                    def v_group(lo, hi):
                        # PE-transposes vTp [h, s] -> v_sb chunks [s, h],
                        # rotating through the proj PSUM slots
                        for vb in range(lo // 512, hi // 512):
                            tps = bps.tile(
                                [128, 4, H], bf16, name=f"vt{vtag}_{vb}",
                                tag="proj", bufs=2,
                            )
                            for t in range(4):
                                nc.tensor.transpose(
                                    tps[:, t, :],
                                    vTp[
                                        0:H,
                                        512 * vb + 128 * t : 512 * vb
                                        + 128 * (t + 1),
                                    ],
                                    identb_sb[0:H, 0:H],
                                )
                            nc.vector.tensor_copy(
                                v_sb[:, 4 * vb : 4 * (vb + 1), 0:H], tps[:]
                            )

                    def chunk_geom(nk, ki):
                        d = ki - (nk - 4)  # >=0 for the 4 diagonal chunks
                        qlo = 0 if d < 0 else KCH * d
                        return d, qlo

                    def emit_st_pair(st8, pair, ki):
                        active = [x for x in pair if ki < st8[x]["nk"]]
                        st2 = bps.tile(
                            [128, 2 * JBLK], f32,
                            name=f"st{vtag}_{pair[0]}_{ki}", tag="st", bufs=2,
                        )
                        p2 = bpool.tile(
                            [128, 2 * JBLK], bf16,
                            name=f"p{vtag}_{pair[0]}_{ki}", tag="p", bufs=10,
                        )
                        diag = []
                        span = []
                        # the two streams' score matmuls run concurrently in
                        # disjoint PE row groups (Kt/Qt rows 64..127 hold the
                        # duplicated head dim, so row group 1 reads the copy)
                        for idx, x in enumerate(active):
                            s = st8[x]
                            d, qlo = chunk_geom(s["nk"], ki)
                            off = JBLK * (x - pair[0])
                            rg = 64 * idx
                            nc.tensor.matmul(
                                st2[:, off + qlo : off + JBLK],
                                Kt[rg : rg + H, KCH * ki : KCH * (ki + 1)],
                                Qt[
                                    rg : rg + H,
                                    JBLK * s["jl"] + qlo : JBLK * (s["jl"] + 1),
                                ],
                                start=True,
                                stop=True,
                                tile_position=(rg, 0),
                            )
                            span.append((off + qlo, off + JBLK))
                            if d >= 0:
                                diag.append((x, off + qlo))
                        lo, hi = span[0][0], span[-1][1]
                        nc.scalar.activation(
                            p2[:, lo:hi],
                            st2[:, lo:hi],
                            mybir.ActivationFunctionType.Exp,
                            bias=zbias[:],
                            scale=float(SCALE / 4096.0),
                        )
                        for x, off in diag:
                            # the stream-final diag mask runs on the DVE so
                            # the PV->epilogue tail skips a gpsimd handoff
                            eng = (
                                nc.vector
                                if ki == st8[x]["nk"] - 1
                                else nc.gpsimd
                            )
                            eng.tensor_mul(
                                p2[:, off : off + KCH], p2[:, off : off + KCH],
                                tril_sb[:],
                            )
                        return p2

                    def emit_pv(st8, pair, x, ki, p2):
                        s = st8[x]
                        d, qlo = chunk_geom(s["nk"], ki)
                        off = JBLK * (x - pair[0])
                        nc.tensor.matmul(
                            s["ot"][:, qlo:JBLK],
                            v_sb[:, ki, :],
                            p2[:, off + qlo : off + JBLK],
                            start=(ki == 0),
                            stop=(ki == s["nk"] - 1),
                        )

                    def epilogue(ot, jl):
                        # O^T [65, 512] -> bf16 -> PE transpose -> divide;
                        # fully pipelined per 128-col piece so the tail
                        # after the last PV round is one piece, not four
                        otf = bpool.tile(
                            [H + 1, JBLK], bf16, name=f"otf{vtag}_{jl}",
                            tag="otf", bufs=2,
                        )
                        nc.vector.tensor_copy(otf[:], ot[0 : H + 1, :])
                        otr = bps.tile(
                            [128, 4, H + 2], bf16, name=f"otr{vtag}_{jl}",
                            tag="proj", bufs=2,
                        )
                        rec = bpool.tile(
                            [128, 4, 1], f32, name=f"rec{vtag}_{jl}", tag="rec",
                            bufs=2,
                        )
                        ost = bpool.tile(
                            [128, 4, H], f32, name=f"ost{vtag}_{jl}", tag="ost",
                            bufs=2,
                        )
                        outap = out.ap()[JBLK * jl : JBLK * (jl + 1), :].rearrange(
                            "(t p) h -> p t h", p=128
                        )
                        for t in range(4):
                            nc.tensor.transpose(
                                otr[:, t, 0 : H + 1],
                                otf[:, 128 * t : 128 * (t + 1)],
                                identb_sb[0 : H + 1, 0 : H + 1],
                            )
                            nc.vector.reciprocal(
                                rec[:, t, :], otr[:, t, H : H + 1]
                            )
                            nc.vector.tensor_scalar_mul(
                                ost[:, t, :], otr[:, t, 0:H], rec[:, t, :]
                            )
                            nc.sync.dma_start(outap[:, t : t + 1, :], ost[:, t : t + 1, :])

                    st8 = {}
                    for jl in range(NJ):
                        jg = jglobals[jl]
                        st8[jl] = {"jl": jl, "jg": jg, "nk": 4 * (jg + 1)}

                    def st_step(pair, pbuf, r):
                        rounds = max(st8[x]["nk"] for x in pair)
                        if 0 <= r < rounds:
                            pbuf[r] = emit_st_pair(st8, pair, r)

                    def pv_step(pair, pbuf, r):
                        if r not in pbuf:
                            return
                        for x in pair:
                            if r < st8[x]["nk"]:
                                emit_pv(st8, pair, x, r, pbuf[r])
                        del pbuf[r]
                        for x in pair:
                            if r == st8[x]["nk"] - 1:
                                epilogue(st8[x]["ot"], x)

                    def alloc_ot(pair):
                        for x in pair:
                            st8[x]["ot"] = bps.tile(
                                [128, JBLK], f32, name=f"ot{vtag}_{x}",
                                tag="ot", bufs=2,
                            )

                    small, big = (0, 1), (2, 3)
                    sr = max(st8[x]["nk"] for x in small)
                    br = max(st8[x]["nk"] for x in big)
                    gA = sr // 8  # phase-A extent in 1024-col K/V groups
                    ngrp = (nblk + 1) // 2

                    # warm the PE clock (HAM un-throttles after ~3.4us of
                    # sustained activity) during the initial DMA wait with
                    # dummy matmuls on the weight tile
                    wps = bps.tile(
                        [128, 128], f32, name=f"warm{vtag}", tag="proj", bufs=2
                    )
                    for _ in range(36):
                        nc.tensor.matmul(
                            wps[:], w8_sb[:, 0, 0, :], w8_sb[:, 0, 0, :],
                            start=True, stop=True,
                        )

                    # phase A: small pair chases the q/k DMA stream
                    alloc_ot(small)
                    proj_group(Qt, qT_sb, 0, 0, 1024)
                    pb = {}
                    for g in range(gA):
                        lo, hi = 1024 * g, 1024 * (g + 1)
                        proj_group(Kt, kT_sb, 1, lo, hi)
                        for i in range(8):
                            st_step(small, pb, 8 * g + i)
                            pv_step(small, pb, 8 * (g - 1) + i)
                        if g == 0:
                            proj_group(Qt, qT_sb, 0, 1024, 2048)
                        proj_group(vTp, vT_sb, 2, lo, hi)
                        v_group(lo, hi)
                    # phase boundary: big pair's early score rounds (Kt is
                    # already projected) interleave with small pair's last
                    # PV rounds and epilogues; big PV follows 8 rounds behind
                    # so at most ~9 p2 tiles are ever in flight
                    alloc_ot(big)
                    pb2 = {}
                    tail = list(range(8 * (gA - 1), sr))
                    bn = 8 * gA
                    for i in range(bn):
                        st_step(big, pb2, i)
                        if i < len(tail):
                            pv_step(small, pb, tail[i])
                        if i >= 8:
                            pv_step(big, pb2, i - 8)
                    for i in range(bn, len(tail)):
                        pv_step(small, pb, tail[i])
                    # phase B: big pair chases the remaining K/V stream
                    cst = bn
                    cpv = max(0, bn - 8)
                    for g in range(gA, ngrp):
                        lo, hi = 1024 * g, min(1024 * (g + 1), kvhi)
                        proj_group(Kt, kT_sb, 1, lo, hi)
                        nhi = min(cst + (hi - lo) // KCH, br)
                        npv = max(0, nhi - 8)
                        for i in range(max(nhi - cst, npv - cpv)):
                            if cst + i < nhi:
                                st_step(big, pb2, cst + i)
                            if cpv + i < npv:
                                pv_step(big, pb2, cpv + i)
                        cst, cpv = nhi, npv
                        proj_group(vTp, vT_sb, 2, lo, hi)
                        v_group(lo, hi)
                    for r in range(cpv, br):
                        pv_step(big, pb2, r)

            with tc.If(pid <= 3) as cmp:
                body(JGLOBALS[0], 0)
            with cmp.Else():
                body(JGLOBALS[1], 1)

    nc.compile()
    return nc


def _get_nc():
    if "nc" not in _CACHE:
        _CACHE["nc"] = _build()
    return _CACHE["nc"]


def _numpy_fallback(query, key, value, Wq, bq, Wk, bk, Wv, bv, mask):
    """Exact reference math in numpy; only used if the mask is not causal."""
    q = np.einsum("bse,he->bsh", query, Wq) + bq
    k = np.einsum("bse,he->bsh", key, Wk) + bk
    v = np.einsum("bse,he->bsh", value, Wv) + bv
    scores = np.einsum("bqh,bkh->bqk", q, k) / np.sqrt(np.float32(query.shape[-1]))
    scores = np.where(np.asarray(mask), scores, -np.inf)
    scores -= scores.max(axis=-1, keepdims=True)
    w = np.exp(scores)
    w /= w.sum(axis=-1, keepdims=True)
    return np.einsum("bqk,bkh->bqh", w, v).astype(np.float32)


def _half_rows(arr_s_first, half):
    """Select this half's query rows (its JGLOBALS blocks) from [S, ...]."""
    return np.concatenate(
        [arr_s_first[JBLK * jg : JBLK * (jg + 1)] for jg in JGLOBALS[half]]
    )


def _prepare_in_maps(query, key, value, Wq, bq, Wk, bk, Wv, bv):
    import ml_dtypes as mld

    F8 = mld.float8_e4m3
    # Weight columns (and biases) are duplicated into partitions 64..127 so
    # the paired score matmuls can read the head dim from either PE row
    # group; each matmul contracts its own 64 rows, so scores stay exact.
    # Q/K weights are scaled by 64 so their fp8 quantization avoids the
    # subnormal range; the 64*64 factor is folded out of the exp scale.
    w81 = np.stack([Wq.T, Wk.T]) * 64.0
    w8 = np.concatenate([w81, w81], axis=-1).astype(F8)
    wv1 = Wv.T
    wv = np.concatenate([wv1, wv1], axis=-1).astype(BF16)
    b1 = np.stack([bq * 64.0, bk * 64.0, bv]).reshape(3, H)
    bqkv = np.concatenate([b1, b1], axis=-1).reshape(3, 2 * H, 1).astype(np.float32)

    def _pack(arrT):
        # [E, X] -> [128, X//1024, 4(e-chunk), 1024] -> [128, 4X] so each
        # partition's data is one contiguous run per 1024-col chunk
        X = arrT.shape[1]
        return np.ascontiguousarray(
            arrT.reshape(4, 128, X // 1024, 1024).transpose(1, 2, 0, 3)
        ).reshape(128, 4 * X)

    kT_b = [_pack(key[b].T.astype(F8)) for b in range(B)]
    vT_b = [_pack(value[b].T.astype(BF16)) for b in range(B)]
    in_maps = []
    for j in range(N_CORES):
        b, half = j % B, j // B
        qslab = _half_rows(query[b], half)
        in_maps.append(
            {
                "qT": _pack(qslab.T.astype(F8)),
                "kT": kT_b[b],
                "vT": vT_b[b],
                "w8": w8,
                "wv": wv,
                "bqkv": bqkv,
            }
        )
    return in_maps


def _assemble(results):
    out = np.empty((B, S, H), np.float32)
    for j in range(N_CORES):
        b, half = j % B, j // B
        r = results[j]["out"]
        for jl, jg in enumerate(JGLOBALS[half]):
            out[b, JBLK * jg : JBLK * (jg + 1)] = r[JBLK * jl : JBLK * (jl + 1)]
    return out


def run(query, key, value, Wq, bq, Wk, bk, Wv, bv, mask, trace=False, **trace_kwargs):
    from concourse.bass_utils import run_bass_kernel_spmd

    mask = np.asarray(mask)
    causal = mask.shape == (1, S, S) and bool(
        np.array_equal(mask[0], np.tril(np.ones((S, S), dtype=bool)))
    )
    if not causal:
        return _numpy_fallback(
            query, key, value, Wq, bq, Wk, bk, Wv, bv, mask
        ), None

    args = [np.asarray(a, np.float32) for a in (query, key, value, Wq, bq, Wk, bk, Wv, bv)]
    nc = _get_nc()
    in_maps = _prepare_in_maps(*args)
    res = run_bass_kernel_spmd(
        nc, in_maps, core_ids=list(range(N_CORES)), trace=trace, **trace_kwargs
    )
    return _assemble(res.results), res


def kernel(query, key, value, Wq, bq, Wk, bk, Wv, bv, mask):
    out, _ = run(query, key, value, Wq, bq, Wk, bk, Wv, bv, mask)
    return out


if __name__ == "__main__":
    rng = np.random.default_rng(0)
    query = rng.standard_normal((B, S, E)).astype(np.float32)
    key = rng.standard_normal((B, S, E)).astype(np.float32)
    value = rng.standard_normal((B, S, E)).astype(np.float32)
    Wq = (rng.standard_normal((H, E)) * 0.02).astype(np.float32)
    Wk = (rng.standard_normal((H, E)) * 0.02).astype(np.float32)
    Wv = (rng.standard_normal((H, E)) * 0.02).astype(np.float32)
    bq = np.zeros(H, np.float32)
    bk = np.zeros(H, np.float32)
    bv = np.zeros(H, np.float32)
    mask = np.tril(np.ones((1, S, S), dtype=bool))
    out = kernel(query, key, value, Wq, bq, Wk, bk, Wv, bv, mask)
    exp = _numpy_fallback(query, key, value, Wq, bq, Wk, bk, Wv, bv, mask)
    err = np.linalg.norm(out - exp) / np.linalg.norm(exp)
    print("self-check rel err:", err)


# revision 32
# speedup vs baseline: 1.0564x; 1.0564x over previous
"""Trainium2 Bass kernel for causal single-head attention with QKV projections.

Problem shape: B=4, S=4096, E=512, H=64 (fp32 inputs, causal mask).

Strategy (8 NeuronCores, data-parallel):
  - core j handles batch j%4; half j//4 of that batch's query rows.
    Half 0 = 512-row blocks {0,1,6,7}, half 1 = {2,3,4,5}: both halves do
    exactly 72 causal key-chunks of score work and 40 softmax rounds, and
    half 1 only ever reads K/V columns [0, 3072).
  - Host pre-transposes Q/K/V slabs to [E, S] layout and casts to bf16 so all
    device matmuls have the contraction dim on partitions.
  - On device: project Qt=[H,Sq], Kt=[H,S], vTp=[H,S] (fp32 biases added
    exactly via per-partition bias) with 1024-wide moving operands, then
    flash-style causal attention with scores kept transposed [k-part, q-free]:
       St = Kt_chunk^T @ Qt  -> exp fused into the PSUM->SBUF evacuation
       O^T (+denominator row) = [v | 1]^T @ P accumulated in PSUM
    No max-subtraction (scores are bounded, |s|<1 after 1/sqrt(E) scaling, so
    exp is safe; softmax is shift-invariant so the result matches reference).
  - The head dim (H=64) is duplicated into partitions 64..127 via
    host-duplicated projection weights so paired streams run their score
    matmuls concurrently in disjoint PE row groups (tile_position row tiling),
    each contracting K=64 exactly, and the PE clock stays un-throttled.
  - All transposes run on the DMA xbar (dma transpose), not the PE:
    v_sb [k,h] chunks come from vTp via sync-engine xbar transposes, and the
    per-512-block epilogue transposes O^T via a bf16 xbar round trip before
    the denominator divide.  PSUM holds exactly 8 banks: proj psum (2) +
    2x double-buffered score super-tile (4) + the pair's two O^T
    accumulators (2).
  - Streams are paired by similar causal length ((0,1) then (2,3)); score
    matmuls run one 1024-col K/V group ahead of the PV matmuls with st/pv
    emission interleaved so the PE never waits on the exp evacuations.
  - Input HBM traffic is issued as 1 MiB chunks, q/k first, inside each
    variant branch; the causal-mask diag multiplies and v_sb memsets run on
    the otherwise-idle GPSIMD engine.
"""

import sys

sys.path.insert(0, "/opt/trn_rl_repo")

import math

import numpy as np
import ml_dtypes

B, S, E, H = 4, 4096, 512, 64
N_CORES = 8
SQ = S // 2  # 2048 query rows per core
JBLK = 512  # query block size
NJ = SQ // JBLK  # 4 query blocks per core
KCH = 128  # key chunk size
JGLOBALS = [[0, 1, 6, 7], [2, 3, 4, 5]]  # global 512-row q-block ids per half
KVEXT = [8, 6]  # 512-col K/V blocks each variant actually reads
SCALE = 1.0 / math.sqrt(float(E))

BF16 = ml_dtypes.bfloat16

_CACHE = {}


def _build():
    import concourse.mybir as mybir
    from concourse import bacc, tile

    f32 = mybir.dt.float32
    bf16 = mybir.dt.bfloat16

    nc = bacc.Bacc(
        "TRN2", target_bir_lowering=False, debug=False, num_devices=N_CORES
    )

    f8 = mybir.dt.float8e4
    i16 = mybir.dt.int16

    # Inputs are host-packed to [128 partitions, chunk(1024 cols), e-chunk,
    # col] order so every input DMA is 128 contiguous runs (cheap issue).
    qT = nc.dram_tensor("qT", [128, 4 * SQ], f8, kind="ExternalInput")
    kT = nc.dram_tensor("kT", [128, 4 * S], f8, kind="ExternalInput")
    vT = nc.dram_tensor("vT", [128, 4 * S], bf16, kind="ExternalInput")
    w8 = nc.dram_tensor("w8", [2, E, 2 * H], f8, kind="ExternalInput")
    wv = nc.dram_tensor("wv", [E, 2 * H], bf16, kind="ExternalInput")
    bqkv = nc.dram_tensor("bqkv", [3, 2 * H, 1], f32, kind="ExternalInput")
    out = nc.dram_tensor("out", [SQ, H], f32, kind="ExternalOutput")

    # Embedded constants: causal block mask (allowed = k <= q) and identity.
    tril_np = np.triu(np.ones((KCH, KCH), np.float32)).astype(BF16)
    trilc = nc.inline_tensor(tril_np, name="trilc")
    ident_np = np.eye(128, dtype=BF16)
    identbc = nc.inline_tensor(ident_np, name="identbc")

    with tile.TileContext(nc) as tc:
        pid = nc.partition_id()
        with (
            tc.tile_pool(name="cpool", bufs=1) as cpool,
            tc.tile_pool(name="ipool", bufs=1) as ipool,
        ):
            # ---- constants (small, issued first on the sync queue) ----
            w8_sb = cpool.tile([128, 2, 4, 2 * H], f8, name="w8_sb")
            nc.sync.dma_start(
                w8_sb[:], w8.ap().rearrange("m (c p) h -> p m c h", p=128)
            )
            wv_sb = cpool.tile([128, 4, 2 * H], bf16, name="wv_sb")
            nc.sync.dma_start(
                wv_sb[:], wv.ap().rearrange("(c p) h -> p c h", p=128)
            )
            b_sb = cpool.tile([2 * H, 3], f32, name="b_sb")
            nc.sync.dma_start(b_sb[:], bqkv.ap().rearrange("m h one -> h (m one)"))
            tril_sb = cpool.tile([KCH, KCH], bf16, name="tril_sb")
            nc.sync.dma_start(tril_sb[:], trilc.ap())
            identb_sb = cpool.tile([128, 128], bf16, name="identb_sb")
            nc.sync.dma_start(identb_sb[:], identbc.ap())
            zbias = cpool.tile([128, 1], f32, name="zbias")
            nc.vector.memset(zbias[:], 0.0)

            # ---- input tiles ([128, chunk, e-chunk, 1024] layout) ----
            qT_sb = ipool.tile([128, 2, 4, 1024], f8, name="qT_sb", tag="qT")
            kT_sb = ipool.tile([128, 4, 4, 1024], f8, name="kT_sb", tag="kT")
            vT_sb = ipool.tile([128, 4, 4, 1024], bf16, name="vT_sb", tag="vT")

            def dma_in(eng, srcd, dst, ch):
                # one 1024-col chunk = 128 contiguous 4KB-8KB runs
                eng.dma_start(
                    dst[:, ch], srcd.ap()[:, 4096 * ch : 4096 * (ch + 1)]
                )

            def body(jglobals, vtag):
                nblk = KVEXT[vtag]  # K/V extent in 512-col blocks
                kvhi = 512 * nblk
                # q/k chunks issue on the scalar HWDGE queue (idle until
                # the first exp), v chunks on sync, so compute starts
                # earliest
                nc.scalar.dma_start(qT_sb[:], qT.ap())
                for ch in range(nblk // 2):
                    dma_in(nc.scalar, kT, kT_sb, ch)
                for ch in range(nblk // 2):
                    dma_in(nc.sync, vT, vT_sb, ch)

                with (
                    tc.tile_pool(name=f"bpool{vtag}", bufs=1) as bpool,
                    tc.tile_pool(name=f"bps{vtag}", bufs=1, space="PSUM") as bps,
                ):
                    Qt = bpool.tile([2 * H, SQ], bf16, name=f"Qt{vtag}")
                    Kt = bpool.tile([2 * H, S], bf16, name=f"Kt{vtag}")
                    vTp = bpool.tile([2 * H, S], bf16, name=f"vTp{vtag}")
                    v_sb = bpool.tile(
                        [128, S // KCH, 128], bf16, name=f"v_sb{vtag}"
                    )
                    # ones column = softmax denominator row; zero padding so
                    # O^T rows H+1..79 (copied for the xbar transpose) are 0
                    nc.gpsimd.memset(v_sb[:, :, H : H + 1], 1.0)
                    nc.gpsimd.memset(v_sb[:, :, H + 1 :], 0.0)

                    def proj_block(dst, src_sb, m, blk):
                        ps = bps.tile(
                            [2 * H, 512], f32, name=f"pj{vtag}_{m}_{blk}",
                            tag="proj", bufs=2,
                        )
                        ch, off = blk // 2, 512 * (blk % 2)
                        if m < 2:
                            # fp8 DoubleRow: each matmul contracts a pair of
                            # 128-row E-chunks (2 fp8 weights per PE cell)
                            for g in range(2):
                                nc.tensor.matmul(
                                    ps[:],
                                    w8_sb[:, m, 2 * g : 2 * g + 2, :],
                                    src_sb[:, ch, 2 * g : 2 * g + 2, off : off + 512],
                                    start=(g == 0),
                                    stop=(g == 1),
                                    perf_mode=mybir.MatmulPerfMode.DoubleRow,
                                )
                        else:
                            for c in range(4):
                                nc.tensor.matmul(
                                    ps[:],
                                    wv_sb[:, c, :],
                                    src_sb[:, ch, c, off : off + 512],
                                    start=(c == 0),
                                    stop=(c == 3),
                                )
                        nc.vector.tensor_scalar_add(
                            dst[:, 512 * blk : 512 * blk + 512], ps[:],
                            b_sb[:, m : m + 1],
                        )

                    def proj_group(dst, src_sb, m, lo, hi):
                        for blk in range(lo // 512, hi // 512):
                            proj_block(dst, src_sb, m, blk)

                    def v_group(lo, hi):
                        # PE-transposes vTp [h, s] -> v_sb chunks [s, h],
                        # rotating through the proj PSUM slots
                        for vb in range(lo // 512, hi // 512):
                            tps = bps.tile(
                                [128, 4, H], bf16, name=f"vt{vtag}_{vb}",
                                tag="proj", bufs=2,
                            )
                            for t in range(4):
                                nc.tensor.transpose(
                                    tps[:, t, :],
                                    vTp[
                                        0:H,
                                        512 * vb + 128 * t : 512 * vb
                                        + 128 * (t + 1),
                                    ],
                                    identb_sb[0:H, 0:H],
                                )
                            nc.vector.tensor_copy(
                                v_sb[:, 4 * vb : 4 * (vb + 1), 0:H], tps[:]
                            )

                    def chunk_geom(nk, ki):
                        d = ki - (nk - 4)  # >=0 for the 4 diagonal chunks
                        qlo = 0 if d < 0 else KCH * d
                        return d, qlo

                    def emit_st_pair(st8, pair, ki):
                        active = [x for x in pair if ki < st8[x]["nk"]]
                        st2 = bps.tile(
                            [128, 2 * JBLK], f32,
                            name=f"st{vtag}_{pair[0]}_{ki}", tag="st", bufs=2,
                        )
                        p2 = bpool.tile(
                            [128, 2 * JBLK], bf16,
                            name=f"p{vtag}_{pair[0]}_{ki}", tag="p", bufs=10,
                        )
                        diag = []
                        span = []
                        # the two streams' score matmuls run concurrently in
                        # disjoint PE row groups (Kt/Qt rows 64..127 hold the
                        # duplicated head dim, so row group 1 reads the copy)
                        for idx, x in enumerate(active):
                            s = st8[x]
                            d, qlo = chunk_geom(s["nk"], ki)
                            off = JBLK * (x - pair[0])
                            rg = 64 * idx
                            nc.tensor.matmul(
                                st2[:, off + qlo : off + JBLK],
                                Kt[rg : rg + H, KCH * ki : KCH * (ki + 1)],
                                Qt[
                                    rg : rg + H,
                                    JBLK * s["jl"] + qlo : JBLK * (s["jl"] + 1),
                                ],
                                start=True,
                                stop=True,
                                tile_position=(rg, 0),
                            )
                            span.append((off + qlo, off + JBLK))
                            if d >= 0:
                                diag.append((x, off + qlo))
                        lo, hi = span[0][0], span[-1][1]
                        nc.scalar.activation(
                            p2[:, lo:hi],
                            st2[:, lo:hi],
                            mybir.ActivationFunctionType.Exp,
                            bias=zbias[:],
                            scale=float(SCALE / 4096.0),
                        )
                        for x, off in diag:
                            nc.gpsimd.tensor_mul(
                                p2[:, off : off + KCH], p2[:, off : off + KCH],
                                tril_sb[:],
                            )
                        return p2

                    def emit_pv(st8, pair, x, ki, p2):
                        s = st8[x]
                        d, qlo = chunk_geom(s["nk"], ki)
                        off = JBLK * (x - pair[0])
                        nc.tensor.matmul(
                            s["ot"][:, qlo:JBLK],
                            v_sb[:, ki, :],
                            p2[:, off + qlo : off + JBLK],
                            start=(ki == 0),
                            stop=(ki == s["nk"] - 1),
                        )

                    def epilogue(ot, jl):
                        # O^T [65, 512] -> bf16 -> PE transpose -> divide
                        otf = bpool.tile(
                            [H + 1, JBLK], bf16, name=f"otf{vtag}_{jl}",
                            tag="otf", bufs=2,
                        )
                        nc.vector.tensor_copy(otf[:], ot[0 : H + 1, :])
                        otr = bps.tile(
                            [128, 4, H + 2], bf16, name=f"otr{vtag}_{jl}",
                            tag="proj", bufs=2,
                        )
                        for t in range(4):
                            nc.tensor.transpose(
                                otr[:, t, 0 : H + 1],
                                otf[:, 128 * t : 128 * (t + 1)],
                                identb_sb[0 : H + 1, 0 : H + 1],
                            )
                        rec = bpool.tile(
                            [128, 4, 1], f32, name=f"rec{vtag}_{jl}", tag="rec",
                            bufs=2,
                        )
                        nc.vector.reciprocal(rec[:], otr[:, :, H : H + 1])
                        ost = bpool.tile(
                            [128, 4, H], f32, name=f"ost{vtag}_{jl}", tag="ost",
                            bufs=2,
                        )
                        for t in range(4):
                            nc.vector.tensor_scalar_mul(
                                ost[:, t, :], otr[:, t, 0:H], rec[:, t, :]
                            )
                        nc.sync.dma_start(
                            out.ap()[JBLK * jl : JBLK * (jl + 1), :].rearrange(
                                "(t p) h -> p t h", p=128
                            ),
                            ost[:],
                        )

                    st8 = {}
                    for jl in range(NJ):
                        jg = jglobals[jl]
                        st8[jl] = {"jl": jl, "jg": jg, "nk": 4 * (jg + 1)}

                    def st_step(pair, pbuf, r):
                        rounds = max(st8[x]["nk"] for x in pair)
                        if 0 <= r < rounds:
                            pbuf[r] = emit_st_pair(st8, pair, r)

                    def pv_step(pair, pbuf, r):
                        if r not in pbuf:
                            return
                        for x in pair:
                            if r < st8[x]["nk"]:
                                emit_pv(st8, pair, x, r, pbuf[r])
                        del pbuf[r]
                        for x in pair:
                            if r == st8[x]["nk"] - 1:
                                epilogue(st8[x]["ot"], x)

                    def alloc_ot(pair):
                        for x in pair:
                            st8[x]["ot"] = bps.tile(
                                [128, JBLK], f32, name=f"ot{vtag}_{x}",
                                tag="ot", bufs=2,
                            )

                    small, big = (0, 1), (2, 3)
                    sr = max(st8[x]["nk"] for x in small)
                    br = max(st8[x]["nk"] for x in big)
                    gA = sr // 8  # phase-A extent in 1024-col K/V groups
                    ngrp = (nblk + 1) // 2

                    # warm the PE clock (HAM un-throttles after ~3.4us of
                    # sustained activity) during the initial DMA wait with
                    # dummy matmuls on the weight tile
                    wps = bps.tile(
                        [128, 128], f32, name=f"warm{vtag}", tag="proj", bufs=2
                    )
                    for _ in range(36):
                        nc.tensor.matmul(
                            wps[:], w8_sb[:, 0, 0, :], w8_sb[:, 0, 0, :],
                            start=True, stop=True,
                        )

                    # phase A: small pair chases the q/k DMA stream
                    alloc_ot(small)
                    proj_group(Qt, qT_sb, 0, 0, 1024)
                    pb = {}
                    for g in range(gA):
                        lo, hi = 1024 * g, 1024 * (g + 1)
                        proj_group(Kt, kT_sb, 1, lo, hi)
                        for i in range(8):
                            st_step(small, pb, 8 * g + i)
                            pv_step(small, pb, 8 * (g - 1) + i)
                        if g == 0:
                            proj_group(Qt, qT_sb, 0, 1024, 2048)
                        proj_group(vTp, vT_sb, 2, lo, hi)
                        v_group(lo, hi)
                    # phase boundary: big pair's early score rounds (Kt is
                    # already projected) interleave with small pair's last
                    # PV rounds and epilogues; big PV follows 8 rounds behind
                    # so at most ~9 p2 tiles are ever in flight
                    alloc_ot(big)
                    pb2 = {}
                    tail = list(range(8 * (gA - 1), sr))
                    bn = 8 * gA
                    for i in range(bn):
                        st_step(big, pb2, i)
                        if i < len(tail):
                            pv_step(small, pb, tail[i])
                        if i >= 8:
                            pv_step(big, pb2, i - 8)
                    for i in range(bn, len(tail)):
                        pv_step(small, pb, tail[i])
                    # phase B: big pair chases the remaining K/V stream
                    cst = bn
                    cpv = max(0, bn - 8)
                    for g in range(gA, ngrp):
                        lo, hi = 1024 * g, min(1024 * (g + 1), kvhi)
                        proj_group(Kt, kT_sb, 1, lo, hi)
                        nhi = min(cst + (hi - lo) // KCH, br)
                        npv = max(0, nhi - 8)
                        for i in range(max(nhi - cst, npv - cpv)):
                            if cst + i < nhi:
                                st_step(big, pb2, cst + i)
                            if cpv + i < npv:
                                pv_step(big, pb2, cpv + i)
                        cst, cpv = nhi, npv
                        proj_group(vTp, vT_sb, 2, lo, hi)
                        v_group(lo, hi)
                    for r in range(cpv, br):
                        pv_step(big, pb2, r)

            with tc.If(pid <= 3) as cmp:
                body(JGLOBALS[0], 0)
            with cmp.Else():
                body(JGLOBALS[1], 1)

    nc.compile()
    return nc


def _get_nc():
    if "nc" not in _CACHE:
        _CACHE["nc"] = _build()
    return _CACHE["nc"]


def _numpy_fallback(query, key, value, Wq, bq, Wk, bk, Wv, bv, mask):
    """Exact reference math in numpy; only used if the mask is not causal."""
    q = np.einsum("bse,he->bsh", query, Wq) + bq
    k = np.einsum("bse,he->bsh", key, Wk) + bk
    v = np.einsum("bse,he->bsh", value, Wv) + bv
    scores = np.einsum("bqh,bkh->bqk", q, k) / np.sqrt(np.float32(query.shape[-1]))
    scores = np.where(np.asarray(mask), scores, -np.inf)
    scores -= scores.max(axis=-1, keepdims=True)
    w = np.exp(scores)
    w /= w.sum(axis=-1, keepdims=True)
    return np.einsum("bqk,bkh->bqh", w, v).astype(np.float32)


def _half_rows(arr_s_first, half):
    """Select this half's query rows (its JGLOBALS blocks) from [S, ...]."""
    return np.concatenate(
        [arr_s_first[JBLK * jg : JBLK * (jg + 1)] for jg in JGLOBALS[half]]
    )


def _prepare_in_maps(query, key, value, Wq, bq, Wk, bk, Wv, bv):
    import ml_dtypes as mld

    F8 = mld.float8_e4m3
    # Weight columns (and biases) are duplicated into partitions 64..127 so
    # the paired score matmuls can read the head dim from either PE row
    # group; each matmul contracts its own 64 rows, so scores stay exact.
    # Q/K weights are scaled by 64 so their fp8 quantization avoids the
    # subnormal range; the 64*64 factor is folded out of the exp scale.
    w81 = np.stack([Wq.T, Wk.T]) * 64.0
    w8 = np.concatenate([w81, w81], axis=-1).astype(F8)
    wv1 = Wv.T
    wv = np.concatenate([wv1, wv1], axis=-1).astype(BF16)
    b1 = np.stack([bq * 64.0, bk * 64.0, bv]).reshape(3, H)
    bqkv = np.concatenate([b1, b1], axis=-1).reshape(3, 2 * H, 1).astype(np.float32)

    def _pack(arrT):
        # [E, X] -> [128, X//1024, 4(e-chunk), 1024] -> [128, 4X] so each
        # partition's data is one contiguous run per 1024-col chunk
        X = arrT.shape[1]
        return np.ascontiguousarray(
            arrT.reshape(4, 128, X // 1024, 1024).transpose(1, 2, 0, 3)
        ).reshape(128, 4 * X)

    kT_b = [_pack(key[b].T.astype(F8)) for b in range(B)]
    vT_b = [_pack(value[b].T.astype(BF16)) for b in range(B)]
    in_maps = []
    for j in range(N_CORES):
        b, half = j % B, j // B
        qslab = _half_rows(query[b], half)
        in_maps.append(
            {
                "qT": _pack(qslab.T.astype(F8)),
                "kT": kT_b[b],
                "vT": vT_b[b],
                "w8": w8,
                "wv": wv,
                "bqkv": bqkv,
            }
        )
    return in_maps


def _assemble(results):
    out = np.empty((B, S, H), np.float32)
    for j in range(N_CORES):
        b, half = j % B, j // B
        r = results[j]["out"]
        for jl, jg in enumerate(JGLOBALS[half]):
            out[b, JBLK * jg : JBLK * (jg + 1)] = r[JBLK * jl : JBLK * (jl + 1)]
    return out


def run(query, key, value, Wq, bq, Wk, bk, Wv, bv, mask, trace=False, **trace_kwargs):
    from concourse.bass_utils import run_bass_kernel_spmd

    mask = np.asarray(mask)
    causal = mask.shape == (1, S, S) and bool(
        np.array_equal(mask[0], np.tril(np.ones((S, S), dtype=bool)))
    )
    if not causal:
        return _numpy_fallback(
            query, key, value, Wq, bq, Wk, bk, Wv, bv, mask
        ), None

    args = [np.asarray(a, np.float32) for a in (query, key, value, Wq, bq, Wk, bk, Wv, bv)]
    nc = _get_nc()
    in_maps = _prepare_in_maps(*args)
    res = run_bass_kernel_spmd(
        nc, in_maps, core_ids=list(range(N_CORES)), trace=trace, **trace_kwargs
    )
    return _assemble(res.results), res


def kernel(query, key, value, Wq, bq, Wk, bk, Wv, bv, mask):
    out, _ = run(query, key, value, Wq, bq, Wk, bk, Wv, bv, mask)
    return out


if __name__ == "__main__":
    rng = np.random.default_rng(0)
    query = rng.standard_normal((B, S, E)).astype(np.float32)
    key = rng.standard_normal((B, S, E)).astype(np.float32)
    value = rng.standard_normal((B, S, E)).astype(np.float32)
    Wq = (rng.standard_normal((H, E)) * 0.02).astype(np.float32)
    Wk = (rng.standard_normal((H, E)) * 0.02).astype(np.float32)
    Wv = (rng.standard_normal((H, E)) * 0.02).astype(np.float32)
    bq = np.zeros(H, np.float32)
    bk = np.zeros(H, np.float32)
    bv = np.zeros(H, np.float32)
    mask = np.tril(np.ones((1, S, S), dtype=bool))
    out = kernel(query, key, value, Wq, bq, Wk, bk, Wv, bv, mask)
    exp = _numpy_fallback(query, key, value, Wq, bq, Wk, bk, Wv, bv, mask)
    err = np.linalg.norm(out - exp) / np.linalg.norm(exp)
    print("self-check rel err:", err)
